# revision 20
# baseline (speedup 1.0000x reference)
"""BiLinearAttention Trainium2 kernel (8 NeuronCores, data-parallel over B).

Math (per batch b, N=128 positions, K=64 keys, D=256, H=8 heads):
  scores[n,k,h] = q[n,:] @ W[h] @ k[n,k,:] + bias[h]
  attn = softmax over the torch-style mixed reshape (B,N,H,K) of scores
  output[n,d]   = sum_{h,kk} attn[n,h,kk] * v[n,kk,d]

Per-core plan: phase1 qW = q@W2 on PE + PE-transposes to qWT[e, n*8+h];
phase2 quad matmuls (4 positions batched in the free dim, 4x redundant
cross-position columns) -> scores [32=(pp,h'), 256=(pp',k)] per quad;
exp+bias on ACT over the full quad tile; the valid diagonal blocks are
extracted with SBUF->SBUF DMAs into E2 [32, 2048]; softmax group sums via
j-reduce + ones-matmuls + reciprocal; normalize on DVE; head-sums g via
strided reduce; block-diag stationary via scatter DMAs; phase3 pair
matmuls g x v (bf16). Host does layout prep/unpack only (no FLOPs).
"""

import numpy as np
import ml_dtypes

B, N, K, D, H = 8, 128, 64, 256, 8
NCORES = 8
NQ = 32               # quads (4 positions each)
KT_CHUNKS = 8         # kt DMA chunks (16 positions each)
V_CHUNKS = 4          # v DMA chunks (16 pairs each)

_CACHE = {}


def _build():
    if "nc" in _CACHE:
        return _CACHE["nc"]
    from contextlib import ExitStack
    import concourse.tile as tile
    from concourse import bacc, mybir

    dt = mybir.dt
    f32 = dt.float32
    bf16 = dt.bfloat16
    AF = mybir.ActivationFunctionType
    AX = mybir.AxisListType

    nc = bacc.Bacc(
        "TRN2",
        target_bir_lowering=False,
        debug=False,
        enable_asserts=False,
        num_devices=NCORES,
    )

    # ---- DRAM I/O (per-core shapes; host pre-packs layouts) ----
    qt_d = nc.dram_tensor("qt", (128, 2, 128), dt.float32r, kind="ExternalInput").ap()
    w2_d = nc.dram_tensor("w2", (128, 2, 2048), dt.float32r, kind="ExternalInput").ap()
    kt_d = nc.dram_tensor("kt", (KT_CHUNKS, 128, 2, 1024), dt.float32r, kind="ExternalInput").ap()
    vv_d = nc.dram_tensor("vv", (V_CHUNKS, 128, 16 * 256), bf16, kind="ExternalInput").ap()
    br_d = nc.dram_tensor("brep", (32, 1), f32, kind="ExternalInput").ap()
    o4_d = nc.dram_tensor("ones4", (32, 4), bf16, kind="ExternalInput").ap()
    ob_d = nc.dram_tensor("onesbc", (4, 32), bf16, kind="ExternalInput").ap()
    id_d = nc.dram_tensor("ident", (128, 128), f32, kind="ExternalInput").ap()

    eraw_d = nc.dram_tensor("eraw", (32, 2048), bf16, kind="ExternalOutput").ap()
    gt2_d = nc.dram_tensor("gt2", (32, 256), bf16, kind="Internal").ap()
    outp_d = nc.dram_tensor("outp", (2, 64 * 256), f32, kind="ExternalOutput").ap()

    ctx = ExitStack()
    tc = ctx.enter_context(tile.TileContext(nc, num_cores=NCORES))

    cons = ctx.enter_context(tc.tile_pool(name="cons", bufs=1))
    work = ctx.enter_context(tc.tile_pool(name="work", bufs=1))

    # ---- persistent SBUF tensors ----
    qt_t = cons.tile([128, 2, 128], dt.float32r, tag="qt")
    kt_ts = [cons.tile([128, 2, 1024], dt.float32r, name=f"kt{i}", tag=f"kt{i}") for i in range(KT_CHUNKS)]
    vv_ts = [cons.tile([128, 16 * 256], bf16, name=f"vv{i}", tag=f"vv{i}") for i in range(V_CHUNKS)]
    br_t = cons.tile([32, 1], f32, tag="br")
    o4_t = cons.tile([32, 4], bf16, tag="o4")
    ob_t = cons.tile([4, 32], bf16, tag="ob")
    id_t = cons.tile([128, 128], f32, tag="id")

    qwt_t = work.tile([128, 2, 1024], dt.float32r, tag="qwt")   # qWT [e, ec, n*8+h]
    e2_t = work.tile([32, 2048], f32, tag="e2")         # diag-extracted exp
    en_t = work.tile([32, 2048], bf16, tag="en")        # normalized
    r_t = work.tile([32, 256], bf16, tag="r")            # j-reduced sums
    si_t = work.tile([4, 256], bf16, tag="si")           # 1/S
    gt_t = work.tile([32, 256], bf16, tag="gt")         # g [(pp,h'), q*8+j]
    bd_t = work.tile([128, 128], bf16, tag="bd")        # block-diag g [(pp2,kk), pp2*64+a]

    with tc.tile_pool(name="p1sb", bufs=1) as p1sb, \
         tc.tile_pool(name="esb", bufs=1) as esb:
        w2_t = p1sb.tile([128, 2, 2048], dt.float32r, tag="w2")
        qw_t = p1sb.tile([128, 2048], f32, tag="qw")    # qW [n, (h,e)]
        e_ts = [esb.tile([32, 4096], f32, name=f"e{i}", tag=f"e{i}") for i in range(2)]

        # ---- input DMAs (phase-1 operands first, vv last) ----
        nc.sync.dma_start(qt_t[:], qt_d)
        nc.sync.dma_start(w2_t[:, :, 0:1024], w2_d[:, :, 0:1024])
        nc.sync.dma_start(w2_t[:, :, 1024:2048], w2_d[:, :, 1024:2048])
        nc.sync.dma_start(br_t[:], br_d)
        nc.sync.dma_start(o4_t[:], o4_d)
        nc.sync.dma_start(ob_t[:], ob_d)
        nc.sync.dma_start(id_t[:], id_d)
        for i in range(KT_CHUNKS):
            nc.sync.dma_start(kt_ts[i][:], kt_d[i])
        for i in range(V_CHUNKS):
            nc.sync.dma_start(vv_ts[i][:], vv_d[i])

        # ---- phase 1: qW[n,(h,e)] = qT.T @ W2, fp32 ----
        with tc.tile_pool(name="ps1", bufs=1, space="PSUM") as ps1:
            qw_ps = ps1.tile([128, 2048], f32, tag="qw_ps")
            for col in range(4):
                for dc in range(2):
                    nc.tensor.matmul(
                        qw_ps[:, col * 512:(col + 1) * 512],
                        qt_t[:, dc, :],
                        w2_t[:, dc, col * 512:(col + 1) * 512],
                        start=(dc == 0),
                        stop=(dc == 1),
                    )
            for col in range(4):
                nc.vector.tensor_copy(
                    qw_t[:, col * 512:(col + 1) * 512],
                    qw_ps[:, col * 512:(col + 1) * 512],
                )

        # ---- phase 1b: transpose qW -> qWT[e, n*8+h] ----
        with tc.tile_pool(name="pst", bufs=4, space="PSUM") as pst:
            for t in range(16):
                h, ec = t // 2, t % 2
                tr_ps = pst.tile([128, 128], f32, tag="tr")
                nc.tensor.transpose(tr_ps[:], qw_t[:, t * 128:(t + 1) * 128], id_t[:])
                nc.vector.tensor_copy(qwt_t[:, ec, h:1024:8], tr_ps[:])

        # ---- phase 2: quad scores + exp; E[(pp,h'), (q, pp', k)] ----
        f32r = dt.float32r
        with tc.tile_pool(name="ps2", bufs=3, space="PSUM") as ps2:
            for qg in range(8):        # 4 quads per PSUM tile
                sc_ps = ps2.tile([32, 1024], f32, tag="sc")
                for ql in range(4):
                    q = qg * 4 + ql
                    ic, qq = q // 4, q % 4
                    for ec in range(2):
                        nc.tensor.matmul(
                            sc_ps[:, ql * 256:(ql + 1) * 256],
                            qwt_t[:, ec, 32 * q:32 * (q + 1)],
                            kt_ts[ic][:, ec, qq * 256:(qq + 1) * 256],
                            start=(ec == 0),
                            stop=(ec == 1),
                        )
                nc.scalar.activation(
                    e_ts[qg // 4][:, (qg % 4) * 1024:(qg % 4 + 1) * 1024],
                    sc_ps[:],
                    AF.Exp,
                    bias=br_t[:, 0:1],
                )

        # ---- extract valid diagonal blocks: E2[(pp,h'), q*64+k] ----
        for hf in range(2):
            for pp in range(4):
                eng = nc.sync if pp % 2 == 0 else nc.scalar
                eng.dma_start(
                    e2_t[8 * pp:8 * pp + 8, hf * 1024:(hf + 1) * 1024]
                    .rearrange("p (q k) -> p q k", k=64),
                    e_ts[hf][8 * pp:8 * pp + 8, :]
                    .rearrange("p (q c k) -> p q c k", c=4, k=64)[:, :, pp, :],
                )

        # ---- softmax sums: S[n, hg] = sum_{h', j} E2 ----
        with tc.tile_pool(name="ps3", bufs=1, space="PSUM") as ps3:
            with nc.allow_low_precision(reason="bf16 softmax sums fine at 2e-2 tol"):
                nc.vector.reduce_sum(
                    r_t[:],
                    e2_t[:].rearrange("p (x j) -> p x j", j=8),
                    axis=AX.X,
                )
            s_ps = ps3.tile([4, 256], f32, tag="s")
            nc.tensor.matmul(s_ps[:], o4_t[:], r_t[:], start=True, stop=True)
            with nc.allow_low_precision(reason="bf16 1/S fine at 2e-2 tol"):
                nc.vector.reciprocal(si_t[:], s_ps[:])
            sbc_ps = ps3.tile([32, 256], f32, tag="sbc")
            nc.tensor.matmul(sbc_ps[:], ob_t[:], si_t[:], start=True, stop=True)
            # normalize: en = e2 * sbc (broadcast over j)
            nc.vector.tensor_mul(
                en_t[:].rearrange("p (x j) -> p x j", j=8),
                e2_t[:].rearrange("p (x j) -> p x j", j=8),
                sbc_ps[:].unsqueeze(-1).broadcast_to((32, 256, 8)),
            )

    nc.scalar.dma_start(eraw_d, en_t[:])

    # ---- g[(pp,h'), 32j+q] = sum_hg en ----
    with nc.allow_low_precision(reason="bf16 g is plenty for 2e-2 tolerance"):
        nc.vector.reduce_sum(
            gt_t[:].rearrange("p (j q) -> p q j", q=32),
            en_t[:]
            .rearrange("p (q hg j) -> p q hg j", hg=8, j=8)
            .transpose([0, 1, 3, 2]),
            axis=AX.X,
        )

    # ---- block-diag stationary bd[(pp2,kk), pp2*64+(s,q)] = g[2(2q+s... ----
    # via DRAM round-trip: one write of gt, two mixed-radix reads
    nc.vector.memset(bd_t[:], 0.0)
    nc.sync.dma_start(gt2_d, gt_t[:])
    for pp2 in range(2):
        for sx in range(2):
            eng = nc.scalar if sx == 0 else nc.sync
            eng.dma_start(
                bd_t[64 * pp2:64 * pp2 + 64, 64 * pp2 + 32 * sx:64 * pp2 + 32 * sx + 32],
                gt2_d[8 * pp2 + 16 * sx:8 * pp2 + 16 * sx + 8, :]
                .rearrange("hp (j q) -> j hp q", j=8),
            )

    # ---- phase 3: output pairs out[pp2, d] @ psum free offset ----
    with tc.tile_pool(name="ps4", bufs=2, space="PSUM") as ps4, \
         tc.tile_pool(name="stg", bufs=2) as stg:
        for grp in range(8):           # 8 pairs per psum tensor
            o_ps = ps4.tile([2, 2048], f32, tag="o")
            for al in range(8):
                a = grp * 8 + al
                c0 = 32 * (a % 2) + a // 2
                nc.tensor.matmul(
                    o_ps[:, al * 256:(al + 1) * 256],
                    bd_t[:, c0:c0 + 65:64],
                    vv_ts[a // 16][:, (a % 16) * 256:(a % 16) * 256 + 256],
                    start=True, stop=True,
                )
            o_sb = stg.tile([2, 2048], f32, tag="osb")
            nc.vector.tensor_copy(o_sb[:, 0:1024], o_ps[:, 0:1024])
            nc.scalar.copy(o_sb[:, 1024:2048], o_ps[:, 1024:2048])
            nc.sync.dma_start(outp_d[:, grp * 2048:(grp + 1) * 2048], o_sb[:])

    ctx.close()
    nc.finalize()
    _CACHE["nc"] = nc
    return nc


def _host_prep(q, k, v, W, b):
    """Per-core input dicts (layout prep only, no math)."""
    bf = ml_dtypes.bfloat16
    w2 = np.ascontiguousarray(W.transpose(1, 0, 2).reshape(D, H * D))  # [d, h*D+e]
    w2_host = np.ascontiguousarray(
        w2.reshape(2, 128, H * D).transpose(1, 0, 2)
    ).astype(np.float32)  # [p, dc, f]
    ones4 = np.zeros((32, 4), bf)
    for pp in range(4):
        ones4[pp * 8:(pp + 1) * 8, pp] = 1.0
    onesbc = np.zeros((4, 32), bf)
    for pp in range(4):
        onesbc[pp, pp * 8:(pp + 1) * 8] = 1.0
    ident = np.eye(128, dtype=np.float32)
    brep = np.tile(b.astype(np.float32), 4)[:, None].copy()

    in_maps = []
    for c in range(NCORES):
        qe = q[c, :, 0, :]                       # (N, D)
        qt = np.ascontiguousarray(qe.T)          # (D, N)
        qt_host = np.ascontiguousarray(qt.reshape(2, 128, 128).transpose(1, 0, 2))
        ktr = np.ascontiguousarray(k[c].reshape(N * K, D).T)  # (D, N*K)
        kt_host = np.ascontiguousarray(
            ktr.reshape(2, 128, KT_CHUNKS, 1024).transpose(2, 1, 0, 3)
        )
        vf = v[c].reshape(N * K, D)              # rows n*64+kk
        vv_host = np.ascontiguousarray(
            vf.reshape(64, 128, D).transpose(1, 0, 2).reshape(128, 64 * D)
        ).astype(bf)
        vv_host = np.ascontiguousarray(
            vv_host.reshape(128, V_CHUNKS, 16 * 256).transpose(1, 0, 2)
        )
        in_maps.append(
            dict(
                qt=qt_host.astype(np.float32),
                w2=w2_host,
                kt=kt_host.astype(np.float32),
                vv=vv_host,
                brep=brep,
                ones4=ones4,
                onesbc=onesbc,
                ident=ident,
            )
        )
    return in_maps


def _host_post(results):
    output = np.empty((B, N, D), np.float32)
    attn = np.empty((B, N, H, K), np.float32)
    for c in range(NCORES):
        om = results[c]
        op = np.asarray(om["outp"])  # (2, 16384)
        output[c] = op.reshape(2, 64, 256).transpose(1, 0, 2).reshape(N, D)
        En = np.asarray(om["eraw"]).astype(np.float32)  # (32, 2048)
        # attn[n=4q+pp, hg, 8j+h'] = En[8pp+h', q*64 + 8*hg + j]
        E5 = En.reshape(4, 8, NQ, 8, 8)  # [pp, h', q, hg, j]
        attn[c] = E5.transpose(2, 0, 3, 4, 1).reshape(N, H, K)
    return output, attn


def kernel(q, k, v, W, b):
    from concourse.bass_utils import run_bass_kernel_spmd

    nc = _build()
    in_maps = _host_prep(
        np.asarray(q, np.float32),
        np.asarray(k, np.float32),
        np.asarray(v, np.float32),
        np.asarray(W, np.float32),
        np.asarray(b, np.float32),
    )
    res = run_bass_kernel_spmd(nc, in_maps, core_ids=list(range(NCORES)))
    return _host_post(res.results)


# revision 22
# speedup vs baseline: 1.0033x; 1.0033x over previous
"""BiLinearAttention Trainium2 kernel (8 NeuronCores, data-parallel over B).

Math (per batch b, N=128 positions, K=64 keys, D=256, H=8 heads):
  scores[n,k,h] = q[n,:] @ W[h] @ k[n,k,:] + bias[h]
  attn = softmax over the torch-style mixed reshape (B,N,H,K) of scores
  output[n,d]   = sum_{h,kk} attn[n,h,kk] * v[n,kk,d]

Per-core plan: phase1 qW = q@W2 on PE + PE-transposes to qWT[e, n*8+h];
phase2 quad matmuls (4 positions batched in the free dim, 4x redundant
cross-position columns) -> scores [32=(pp,h'), 256=(pp',k)] per quad;
exp+bias on ACT over the full quad tile; the valid diagonal blocks are
extracted with SBUF->SBUF DMAs into E2 [32, 2048]; softmax group sums via
j-reduce + ones-matmuls + reciprocal; normalize on DVE; head-sums g via
strided reduce; block-diag stationary via scatter DMAs; phase3 pair
matmuls g x v (bf16). Host does layout prep/unpack only (no FLOPs).
"""

import numpy as np
import ml_dtypes

B, N, K, D, H = 8, 128, 64, 256, 8
NCORES = 8
NQ = 32               # quads (4 positions each)
KT_CHUNKS = 8         # kt DMA chunks (16 positions each)
V_CHUNKS = 4          # v DMA chunks (16 pairs each)

_CACHE = {}


def _build():
    if "nc" in _CACHE:
        return _CACHE["nc"]
    from contextlib import ExitStack
    import concourse.tile as tile
    from concourse import bacc, mybir

    dt = mybir.dt
    f32 = dt.float32
    bf16 = dt.bfloat16
    AF = mybir.ActivationFunctionType
    AX = mybir.AxisListType

    nc = bacc.Bacc(
        "TRN2",
        target_bir_lowering=False,
        debug=False,
        enable_asserts=False,
        num_devices=NCORES,
    )

    # ---- DRAM I/O (per-core shapes; host pre-packs layouts) ----
    qt_d = nc.dram_tensor("qt", (128, 2, 128), dt.float32r, kind="ExternalInput").ap()
    w2_d = nc.dram_tensor("w2", (128, 2, 2048), dt.float32r, kind="ExternalInput").ap()
    kt_d = nc.dram_tensor("kt", (KT_CHUNKS, 128, 2, 1024), dt.float32r, kind="ExternalInput").ap()
    vv_d = nc.dram_tensor("vv", (V_CHUNKS, 128, 16 * 256), bf16, kind="ExternalInput").ap()
    br_d = nc.dram_tensor("brep", (32, 1), f32, kind="ExternalInput").ap()
    o4_d = nc.dram_tensor("ones4", (32, 4), bf16, kind="ExternalInput").ap()
    ob_d = nc.dram_tensor("onesbc", (4, 32), bf16, kind="ExternalInput").ap()
    id_d = nc.dram_tensor("ident", (128, 128), f32, kind="ExternalInput").ap()

    eraw_d = nc.dram_tensor("eraw", (32, 2048), bf16, kind="ExternalOutput").ap()
    gt2_d = nc.dram_tensor("gt2", (32, 256), bf16, kind="Internal").ap()
    outp_d = nc.dram_tensor("outp", (2, 64 * 256), f32, kind="ExternalOutput").ap()

    ctx = ExitStack()
    tc = ctx.enter_context(tile.TileContext(nc, num_cores=NCORES))

    cons = ctx.enter_context(tc.tile_pool(name="cons", bufs=1))
    work = ctx.enter_context(tc.tile_pool(name="work", bufs=1))

    # ---- persistent SBUF tensors ----
    qt_t = cons.tile([128, 2, 128], dt.float32r, tag="qt")
    kt_ts = [cons.tile([128, 2, 1024], dt.float32r, name=f"kt{i}", tag=f"kt{i}") for i in range(KT_CHUNKS)]
    vv_ts = [cons.tile([128, 16 * 256], bf16, name=f"vv{i}", tag=f"vv{i}") for i in range(V_CHUNKS)]
    br_t = cons.tile([32, 1], f32, tag="br")
    o4_t = cons.tile([32, 4], bf16, tag="o4")
    ob_t = cons.tile([4, 32], bf16, tag="ob")
    id_t = cons.tile([128, 128], f32, tag="id")

    qwt_t = work.tile([128, 2, 1024], dt.float32r, tag="qwt")   # qWT [e, ec, n*8+h]
    e2_t = work.tile([32, 2048], f32, tag="e2")         # diag-extracted exp
    en_t = work.tile([32, 2048], bf16, tag="en")        # normalized
    r_t = work.tile([32, 256], bf16, tag="r")            # j-reduced sums
    si_t = work.tile([4, 256], bf16, tag="si")           # 1/S
    gt_t = work.tile([32, 256], bf16, tag="gt")         # g [(pp,h'), q*8+j]
    bd_t = work.tile([128, 128], bf16, tag="bd")        # block-diag g [(pp2,kk), pp2*64+a]

    with tc.tile_pool(name="p1sb", bufs=1) as p1sb, \
         tc.tile_pool(name="esb", bufs=1) as esb:
        w2_t = p1sb.tile([128, 2, 2048], dt.float32r, tag="w2")
        qw_t = p1sb.tile([128, 2048], f32, tag="qw")    # qW [n, (h,e)]
        e_ts = [esb.tile([32, 4096], f32, name=f"e{i}", tag=f"e{i}") for i in range(2)]

        # ---- input DMAs (phase-1 operands first, vv last) ----
        nc.sync.dma_start(qt_t[:], qt_d)
        nc.sync.dma_start(w2_t[:, :, 0:1024], w2_d[:, :, 0:1024])
        nc.sync.dma_start(w2_t[:, :, 1024:2048], w2_d[:, :, 1024:2048])
        nc.sync.dma_start(br_t[:], br_d)
        nc.sync.dma_start(o4_t[:], o4_d)
        nc.sync.dma_start(ob_t[:], ob_d)
        nc.sync.dma_start(id_t[:], id_d)
        for i in range(KT_CHUNKS):
            nc.sync.dma_start(kt_ts[i][:], kt_d[i])
        for i in range(V_CHUNKS):
            nc.sync.dma_start(vv_ts[i][:], vv_d[i])

        # ---- phase 1: qW[n,(h,e)] = qT.T @ W2, fp32 ----
        with tc.tile_pool(name="ps1", bufs=1, space="PSUM") as ps1:
            qw_ps = ps1.tile([128, 2048], f32, tag="qw_ps")
            for col in range(4):
                for dc in range(2):
                    nc.tensor.matmul(
                        qw_ps[:, col * 512:(col + 1) * 512],
                        qt_t[:, dc, :],
                        w2_t[:, dc, col * 512:(col + 1) * 512],
                        start=(dc == 0),
                        stop=(dc == 1),
                    )
            for col in range(4):
                nc.vector.tensor_copy(
                    qw_t[:, col * 512:(col + 1) * 512],
                    qw_ps[:, col * 512:(col + 1) * 512],
                )

        # ---- phase 1b: transpose qW -> qWT[e, n*8+h] ----
        with tc.tile_pool(name="pst", bufs=4, space="PSUM") as pst:
            for t in range(16):
                h, ec = t // 2, t % 2
                tr_ps = pst.tile([128, 128], f32, tag="tr")
                nc.tensor.transpose(tr_ps[:], qw_t[:, t * 128:(t + 1) * 128], id_t[:])
                nc.vector.tensor_copy(qwt_t[:, ec, h:1024:8], tr_ps[:])

        # ---- phase 2: quad scores + exp; E[(pp,h'), (q, pp', k)] ----
        f32r = dt.float32r
        with tc.tile_pool(name="ps2", bufs=3, space="PSUM") as ps2:
            for qg in range(8):        # 4 quads per PSUM tile
                sc_ps = ps2.tile([32, 1024], f32, tag="sc")
                for ql in range(4):
                    q = qg * 4 + ql
                    ic, qq = q // 4, q % 4
                    for ec in range(2):
                        nc.tensor.matmul(
                            sc_ps[:, ql * 256:(ql + 1) * 256],
                            qwt_t[:, ec, 32 * q:32 * (q + 1)],
                            kt_ts[ic][:, ec, qq * 256:(qq + 1) * 256],
                            start=(ec == 0),
                            stop=(ec == 1),
                        )
                nc.scalar.activation(
                    e_ts[qg // 4][:, (qg % 4) * 1024:(qg % 4 + 1) * 1024],
                    sc_ps[:],
                    AF.Exp,
                    bias=br_t[:, 0:1],
                )

        # ---- extract valid diagonal blocks: E2[(pp,h'), q*64+k] ----
        for hf in range(2):
            for pp in range(4):
                eng = nc.sync if pp % 2 == 0 else nc.scalar
                eng.dma_start(
                    e2_t[8 * pp:8 * pp + 8, hf * 1024:(hf + 1) * 1024]
                    .rearrange("p (q k) -> p q k", k=64),
                    e_ts[hf][8 * pp:8 * pp + 8, :]
                    .rearrange("p (q c k) -> p q c k", c=4, k=64)[:, :, pp, :],
                )

        # ---- softmax sums: S[n, hg] = sum_{h', j} E2 ----
        with tc.tile_pool(name="ps3", bufs=1, space="PSUM") as ps3:
            with nc.allow_low_precision(reason="bf16 softmax sums fine at 2e-2 tol"):
                nc.vector.reduce_sum(
                    r_t[:],
                    e2_t[:].rearrange("p (x j) -> p x j", j=8),
                    axis=AX.X,
                )
            s_ps = ps3.tile([4, 256], f32, tag="s")
            nc.tensor.matmul(s_ps[:], o4_t[:], r_t[:], start=True, stop=True)
            with nc.allow_low_precision(reason="bf16 1/S fine at 2e-2 tol"):
                nc.vector.reciprocal(si_t[:], s_ps[:])
            sbc_ps = ps3.tile([32, 256], f32, tag="sbc")
            nc.tensor.matmul(sbc_ps[:], ob_t[:], si_t[:], start=True, stop=True)
            # normalize: en = e2 * sbc (broadcast over j)
            nc.vector.tensor_mul(
                en_t[:].rearrange("p (x j) -> p x j", j=8),
                e2_t[:].rearrange("p (x j) -> p x j", j=8),
                sbc_ps[:].unsqueeze(-1).broadcast_to((32, 256, 8)),
            )

    nc.scalar.dma_start(eraw_d, en_t[:])

    # ---- g[(pp,h'), 32j+q] = sum_hg en ----
    with nc.allow_low_precision(reason="bf16 g is plenty for 2e-2 tolerance"):
        nc.vector.reduce_sum(
            gt_t[:].rearrange("p (j q) -> p q j", q=32),
            en_t[:]
            .rearrange("p (q hg j) -> p q hg j", hg=8, j=8)
            .transpose([0, 1, 3, 2]),
            axis=AX.X,
        )

    # ---- block-diag stationary bd[(pp2,kk), pp2*64+(s,q)] = g[2(2q+s... ----
    # via DRAM round-trip: one write of gt, two mixed-radix reads
    nc.vector.memset(bd_t[:], 0.0)
    nc.sync.dma_start(gt2_d, gt_t[:])
    for pp2 in range(2):
        for sx in range(2):
            eng = nc.scalar if sx == 0 else nc.sync
            eng.dma_start(
                bd_t[64 * pp2:64 * pp2 + 64, 64 * pp2 + 32 * sx:64 * pp2 + 32 * sx + 32],
                gt2_d[8 * pp2 + 16 * sx:8 * pp2 + 16 * sx + 8, :]
                .rearrange("hp (j q) -> j hp q", j=8),
            )

    # ---- phase 3: output pairs out[pp2, d] @ psum free offset ----
    with tc.tile_pool(name="ps4", bufs=2, space="PSUM") as ps4, \
         tc.tile_pool(name="stg", bufs=2) as stg:
        for grp in range(8):           # 8 pairs per psum tensor
            o_ps = ps4.tile([2, 2048], f32, tag="o")
            for al in range(8):
                a = grp * 8 + al
                c0 = 32 * (a % 2) + a // 2
                nc.tensor.matmul(
                    o_ps[:, al * 256:(al + 1) * 256],
                    bd_t[:, c0:c0 + 65:64],
                    vv_ts[a // 16][:, (a % 16) * 256:(a % 16) * 256 + 256],
                    start=True, stop=True,
                )
            o_sb = stg.tile([2, 2048], f32, tag="osb")
            nc.vector.tensor_copy(o_sb[:, 0:1024], o_ps[:, 0:1024])
            nc.scalar.copy(o_sb[:, 1024:2048], o_ps[:, 1024:2048])
            nc.sync.dma_start(outp_d[:, grp * 2048:(grp + 1) * 2048], o_sb[:])

    ctx.close()
    nc.finalize()
    _CACHE["nc"] = nc
    return nc


def _host_prep(q, k, v, W, b):
    """Per-core input dicts (layout prep only, no math)."""
    bf = ml_dtypes.bfloat16
    w2 = np.ascontiguousarray(W.transpose(1, 0, 2).reshape(D, H * D))  # [d, h*D+e]
    w2_host = np.ascontiguousarray(
        w2.reshape(2, 128, H * D).transpose(1, 0, 2)
    ).astype(np.float32)  # [p, dc, f]
    ones4 = np.zeros((32, 4), bf)
    for pp in range(4):
        ones4[pp * 8:(pp + 1) * 8, pp] = 1.0
    onesbc = np.zeros((4, 32), bf)
    for pp in range(4):
        onesbc[pp, pp * 8:(pp + 1) * 8] = 1.0
    ident = np.eye(128, dtype=np.float32)
    brep = np.tile(b.astype(np.float32), 4)[:, None].copy()

    in_maps = []
    for c in range(NCORES):
        qe = q[c, :, 0, :]                       # (N, D)
        qt = np.ascontiguousarray(qe.T)          # (D, N)
        qt_host = np.ascontiguousarray(qt.reshape(2, 128, 128).transpose(1, 0, 2))
        ktr = np.ascontiguousarray(k[c].reshape(N * K, D).T)  # (D, N*K)
        kt_host = np.ascontiguousarray(
            ktr.reshape(2, 128, KT_CHUNKS, 1024).transpose(2, 1, 0, 3)
        )
        vf = v[c].reshape(N * K, D)              # rows n*64+kk
        vv_host = np.ascontiguousarray(
            vf.reshape(64, 128, D).transpose(1, 0, 2).reshape(128, 64 * D)
        ).astype(bf)
        vv_host = np.ascontiguousarray(
            vv_host.reshape(128, V_CHUNKS, 16 * 256).transpose(1, 0, 2)
        )
        in_maps.append(
            dict(
                qt=qt_host.astype(np.float32),
                w2=w2_host,
                kt=kt_host.astype(np.float32),
                vv=vv_host,
                brep=brep,
                ones4=ones4,
                onesbc=onesbc,
                ident=ident,
            )
        )
    return in_maps


def _host_post(results):
    output = np.empty((B, N, D), np.float32)
    attn = np.empty((B, N, H, K), np.float32)
    for c in range(NCORES):
        om = results[c]
        op = np.asarray(om["outp"])  # (2, 16384)
        output[c] = op.reshape(2, 64, 256).transpose(1, 0, 2).reshape(N, D)
        En = np.asarray(om["eraw"]).astype(np.float32)  # (32, 2048)
        # attn[n=4q+pp, hg, 8j+h'] = En[8pp+h', q*64 + 8*hg + j]
        E5 = En.reshape(4, 8, NQ, 8, 8)  # [pp, h', q, hg, j]
        attn[c] = E5.transpose(2, 0, 3, 4, 1).reshape(N, H, K)
    return output, attn


def kernel(q, k, v, W, b):
    from concourse.bass_utils import run_bass_kernel_spmd

    nc = _build()
    in_maps = _host_prep(
        np.asarray(q, np.float32),
        np.asarray(k, np.float32),
        np.asarray(v, np.float32),
        np.asarray(W, np.float32),
        np.asarray(b, np.float32),
    )
    res = run_bass_kernel_spmd(nc, in_maps, core_ids=list(range(NCORES)))
    return _host_post(res.results)


# revision 23
# speedup vs baseline: 1.0573x; 1.0538x over previous
"""BiLinearAttention Trainium2 kernel (8 NeuronCores, data-parallel over B).

Math (per batch b, N=128 positions, K=64 keys, D=256, H=8 heads):
  scores[n,k,h] = q[n,:] @ W[h] @ k[n,k,:] + bias[h]
  attn = softmax over the torch-style mixed reshape (B,N,H,K) of scores
  output[n,d]   = sum_{h,kk} attn[n,h,kk] * v[n,kk,d]

Per-core plan: phase1 qW = q@W2 on PE + PE-transposes to qWT[e, n*8+h];
phase2 quad matmuls (4 positions batched in the free dim, 4x redundant
cross-position columns) -> scores [32=(pp,h'), 256=(pp',k)] per quad;
exp+bias on ACT over the full quad tile; the valid diagonal blocks are
extracted with SBUF->SBUF DMAs into E2 [32, 2048]; softmax group sums via
j-reduce + ones-matmuls + reciprocal; normalize on DVE; head-sums g via
strided reduce; block-diag stationary via scatter DMAs; phase3 pair
matmuls g x v (bf16). Host does layout prep/unpack only (no FLOPs).
"""

import numpy as np
import ml_dtypes

B, N, K, D, H = 8, 128, 64, 256, 8
NCORES = 8
NQ = 32               # quads (4 positions each)
KT_CHUNKS = 8         # kt DMA chunks (16 positions each)
V_CHUNKS = 4          # v DMA chunks (16 pairs each)

_CACHE = {}


def _build():
    if "nc" in _CACHE:
        return _CACHE["nc"]
    from contextlib import ExitStack
    import concourse.tile as tile
    from concourse import bacc, mybir

    dt = mybir.dt
    f32 = dt.float32
    bf16 = dt.bfloat16
    AF = mybir.ActivationFunctionType
    AX = mybir.AxisListType

    nc = bacc.Bacc(
        "TRN2",
        target_bir_lowering=False,
        debug=False,
        enable_asserts=False,
        num_devices=NCORES,
    )

    # ---- DRAM I/O (per-core shapes; host pre-packs layouts) ----
    qt_d = nc.dram_tensor("qt", (128, 2, 128), dt.float32r, kind="ExternalInput").ap()
    w2_d = nc.dram_tensor("w2", (128, 2, 2048), dt.float32r, kind="ExternalInput").ap()
    kt_d = nc.dram_tensor("kt", (KT_CHUNKS, 128, 2, 1024), dt.float32r, kind="ExternalInput").ap()
    vv_d = nc.dram_tensor("vv", (V_CHUNKS, 128, 16 * 256), bf16, kind="ExternalInput").ap()
    br_d = nc.dram_tensor("brep", (32, 1), f32, kind="ExternalInput").ap()
    o4_d = nc.dram_tensor("ones4", (32, 4), bf16, kind="ExternalInput").ap()
    ob_d = nc.dram_tensor("onesbc", (4, 32), bf16, kind="ExternalInput").ap()
    id_d = nc.dram_tensor("ident", (128, 128), f32, kind="ExternalInput").ap()

    eraw_d = nc.dram_tensor("eraw", (32, 2048), bf16, kind="ExternalOutput").ap()
    gt2_d = nc.dram_tensor("gt2", (32, 256), bf16, kind="Internal").ap()
    outp_d = nc.dram_tensor("outp", (2, 64 * 256), f32, kind="ExternalOutput").ap()

    ctx = ExitStack()
    tc = ctx.enter_context(tile.TileContext(nc, num_cores=NCORES))

    cons = ctx.enter_context(tc.tile_pool(name="cons", bufs=1))
    work = ctx.enter_context(tc.tile_pool(name="work", bufs=1))

    # ---- persistent SBUF tensors ----
    qt_t = cons.tile([128, 2, 128], dt.float32r, tag="qt")
    kt_ts = [cons.tile([128, 2, 1024], dt.float32r, name=f"kt{i}", tag=f"kt{i}") for i in range(KT_CHUNKS)]
    vv_ts = [cons.tile([128, 16 * 256], bf16, name=f"vv{i}", tag=f"vv{i}") for i in range(V_CHUNKS)]
    br_t = cons.tile([32, 1], f32, tag="br")
    o4_t = cons.tile([32, 4], bf16, tag="o4")
    ob_t = cons.tile([4, 32], bf16, tag="ob")
    id_t = cons.tile([128, 128], f32, tag="id")

    qwt_t = work.tile([128, 2, 1024], dt.float32r, tag="qwt")   # qWT [e, ec, n*8+h]
    e2_t = work.tile([32, 2048], f32, tag="e2")         # diag-extracted exp
    en_t = work.tile([32, 2048], bf16, tag="en")        # normalized
    r_t = work.tile([32, 256], bf16, tag="r")            # j-reduced sums
    si_t = work.tile([4, 256], bf16, tag="si")           # 1/S
    gt_t = work.tile([32, 256], bf16, tag="gt")         # g [(pp,h'), q*8+j]
    bd_t = work.tile([128, 128], bf16, tag="bd")        # block-diag g [(pp2,kk), pp2*64+a]

    with tc.tile_pool(name="p1sb", bufs=1) as p1sb, \
         tc.tile_pool(name="esb", bufs=1) as esb:
        w2_t = p1sb.tile([128, 2, 2048], dt.float32r, tag="w2")
        qw_t = p1sb.tile([128, 2048], f32, tag="qw")    # qW [n, (h,e)]
        e_ts = [esb.tile([32, 4096], f32, name=f"e{i}", tag=f"e{i}") for i in range(2)]

        # ---- input DMAs (phase-1 operands first, vv last) ----
        nc.sync.dma_start(qt_t[:], qt_d)
        nc.sync.dma_start(w2_t[:, :, 0:1024], w2_d[:, :, 0:1024])
        nc.sync.dma_start(w2_t[:, :, 1024:2048], w2_d[:, :, 1024:2048])
        nc.sync.dma_start(br_t[:], br_d)
        nc.sync.dma_start(o4_t[:], o4_d)
        nc.sync.dma_start(ob_t[:], ob_d)
        nc.sync.dma_start(id_t[:], id_d)
        for i in range(KT_CHUNKS):
            nc.sync.dma_start(kt_ts[i][:], kt_d[i])
        for i in range(V_CHUNKS):
            nc.sync.dma_start(vv_ts[i][:], vv_d[i])

        # ---- phase 1: qW[n,(h,e)] = qT.T @ W2, fp32 ----
        with tc.tile_pool(name="ps1", bufs=1, space="PSUM") as ps1:
            qw_ps = ps1.tile([128, 2048], f32, tag="qw_ps")
            for col in range(4):
                for dc in range(2):
                    nc.tensor.matmul(
                        qw_ps[:, col * 512:(col + 1) * 512],
                        qt_t[:, dc, :],
                        w2_t[:, dc, col * 512:(col + 1) * 512],
                        start=(dc == 0),
                        stop=(dc == 1),
                    )
            for col in range(4):
                nc.vector.tensor_copy(
                    qw_t[:, col * 512:(col + 1) * 512],
                    qw_ps[:, col * 512:(col + 1) * 512],
                )

        # ---- phase 1b: transpose qW -> qWT[e, n*8+h] ----
        with tc.tile_pool(name="pst", bufs=4, space="PSUM") as pst:
            for t in range(16):
                h, ec = t // 2, t % 2
                tr_ps = pst.tile([128, 128], f32, tag="tr")
                nc.tensor.transpose(tr_ps[:], qw_t[:, t * 128:(t + 1) * 128], id_t[:])
                nc.vector.tensor_copy(qwt_t[:, ec, h:1024:8], tr_ps[:])

        # ---- phase 2: quad scores + exp; E[(pp,h'), (q, pp', k)] ----
        f32r = dt.float32r
        with tc.tile_pool(name="ps2", bufs=4, space="PSUM") as ps2:
            for qg in range(8):        # 4 quads per PSUM tile
                sc_ps = ps2.tile([32, 1024], f32, tag="sc")
                for ql in range(4):
                    q = qg * 4 + ql
                    ic, qq = q // 4, q % 4
                    for ec in range(2):
                        nc.tensor.matmul(
                            sc_ps[:, ql * 256:(ql + 1) * 256],
                            qwt_t[:, ec, 32 * q:32 * (q + 1)],
                            kt_ts[ic][:, ec, qq * 256:(qq + 1) * 256],
                            start=(ec == 0),
                            stop=(ec == 1),
                        )
                nc.scalar.activation(
                    e_ts[qg // 4][:, (qg % 4) * 1024:(qg % 4 + 1) * 1024],
                    sc_ps[:],
                    AF.Exp,
                    bias=br_t[:, 0:1],
                )

        # ---- extract valid diagonal blocks: E2[(pp,h'), q*64+k] ----
        for hf in range(2):
            for pp in range(4):
                eng = nc.sync if pp % 2 == 0 else nc.scalar
                eng.dma_start(
                    e2_t[8 * pp:8 * pp + 8, hf * 1024:(hf + 1) * 1024]
                    .rearrange("p (q k) -> p q k", k=64),
                    e_ts[hf][8 * pp:8 * pp + 8, :]
                    .rearrange("p (q c k) -> p q c k", c=4, k=64)[:, :, pp, :],
                )

        # ---- softmax sums: S[n, hg] = sum_{h', j} E2 ----
        with tc.tile_pool(name="ps3", bufs=1, space="PSUM") as ps3:
            with nc.allow_low_precision(reason="bf16 softmax sums fine at 2e-2 tol"):
                nc.vector.reduce_sum(
                    r_t[:],
                    e2_t[:].rearrange("p (x j) -> p x j", j=8),
                    axis=AX.X,
                )
            s_ps = ps3.tile([4, 256], f32, tag="s")
            nc.tensor.matmul(s_ps[:], o4_t[:], r_t[:], start=True, stop=True)
            with nc.allow_low_precision(reason="bf16 1/S fine at 2e-2 tol"):
                nc.vector.reciprocal(si_t[:], s_ps[:])
            sbc_ps = ps3.tile([32, 256], f32, tag="sbc")
            nc.tensor.matmul(sbc_ps[:], ob_t[:], si_t[:], start=True, stop=True)
            # normalize: en = e2 * sbc (broadcast over j)
            nc.vector.tensor_mul(
                en_t[:].rearrange("p (x j) -> p x j", j=8),
                e2_t[:].rearrange("p (x j) -> p x j", j=8),
                sbc_ps[:].unsqueeze(-1).broadcast_to((32, 256, 8)),
            )

    nc.scalar.dma_start(eraw_d, en_t[:])

    # ---- g[(pp,h'), 32j+q] = sum_hg en ----
    with nc.allow_low_precision(reason="bf16 g is plenty for 2e-2 tolerance"):
        nc.vector.reduce_sum(
            gt_t[:].rearrange("p (j q) -> p q j", q=32),
            en_t[:]
            .rearrange("p (q hg j) -> p q hg j", hg=8, j=8)
            .transpose([0, 1, 3, 2]),
            axis=AX.X,
        )

    # ---- block-diag stationary bd[(pp2,kk), pp2*64+(s,q)] = g[2(2q+s... ----
    # via DRAM round-trip: one write of gt, two mixed-radix reads
    nc.vector.memset(bd_t[:], 0.0)
    nc.sync.dma_start(gt2_d, gt_t[:])
    for pp2 in range(2):
        for sx in range(2):
            eng = nc.scalar if sx == 0 else nc.sync
            eng.dma_start(
                bd_t[64 * pp2:64 * pp2 + 64, 64 * pp2 + 32 * sx:64 * pp2 + 32 * sx + 32],
                gt2_d[8 * pp2 + 16 * sx:8 * pp2 + 16 * sx + 8, :]
                .rearrange("hp (j q) -> j hp q", j=8),
            )

    # ---- phase 3: output pairs out[pp2, d] @ psum free offset ----
    with tc.tile_pool(name="ps4", bufs=2, space="PSUM") as ps4, \
         tc.tile_pool(name="stg", bufs=3) as stg:
        for grp in range(8):           # 8 pairs per psum tensor
            o_ps = ps4.tile([2, 2048], f32, tag="o")
            for al in range(8):
                a = grp * 8 + al
                c0 = 32 * (a % 2) + a // 2
                nc.tensor.matmul(
                    o_ps[:, al * 256:(al + 1) * 256],
                    bd_t[:, c0:c0 + 65:64],
                    vv_ts[a // 16][:, (a % 16) * 256:(a % 16) * 256 + 256],
                    start=True, stop=True,
                )
            o_sb = stg.tile([2, 2048], f32, tag="osb")
            nc.vector.tensor_copy(o_sb[:, 0:1024], o_ps[:, 0:1024])
            nc.scalar.copy(o_sb[:, 1024:2048], o_ps[:, 1024:2048])
            nc.sync.dma_start(outp_d[:, grp * 2048:(grp + 1) * 2048], o_sb[:])

    ctx.close()
    nc.finalize()
    _CACHE["nc"] = nc
    return nc


def _host_prep(q, k, v, W, b):
    """Per-core input dicts (layout prep only, no math)."""
    bf = ml_dtypes.bfloat16
    w2 = np.ascontiguousarray(W.transpose(1, 0, 2).reshape(D, H * D))  # [d, h*D+e]
    w2_host = np.ascontiguousarray(
        w2.reshape(2, 128, H * D).transpose(1, 0, 2)
    ).astype(np.float32)  # [p, dc, f]
    ones4 = np.zeros((32, 4), bf)
    for pp in range(4):
        ones4[pp * 8:(pp + 1) * 8, pp] = 1.0
    onesbc = np.zeros((4, 32), bf)
    for pp in range(4):
        onesbc[pp, pp * 8:(pp + 1) * 8] = 1.0
    ident = np.eye(128, dtype=np.float32)
    brep = np.tile(b.astype(np.float32), 4)[:, None].copy()

    in_maps = []
    for c in range(NCORES):
        qe = q[c, :, 0, :]                       # (N, D)
        qt = np.ascontiguousarray(qe.T)          # (D, N)
        qt_host = np.ascontiguousarray(qt.reshape(2, 128, 128).transpose(1, 0, 2))
        ktr = np.ascontiguousarray(k[c].reshape(N * K, D).T)  # (D, N*K)
        kt_host = np.ascontiguousarray(
            ktr.reshape(2, 128, KT_CHUNKS, 1024).transpose(2, 1, 0, 3)
        )
        vf = v[c].reshape(N * K, D)              # rows n*64+kk
        vv_host = np.ascontiguousarray(
            vf.reshape(64, 128, D).transpose(1, 0, 2).reshape(128, 64 * D)
        ).astype(bf)
        vv_host = np.ascontiguousarray(
            vv_host.reshape(128, V_CHUNKS, 16 * 256).transpose(1, 0, 2)
        )
        in_maps.append(
            dict(
                qt=qt_host.astype(np.float32),
                w2=w2_host,
                kt=kt_host.astype(np.float32),
                vv=vv_host,
                brep=brep,
                ones4=ones4,
                onesbc=onesbc,
                ident=ident,
            )
        )
    return in_maps


def _host_post(results):
    output = np.empty((B, N, D), np.float32)
    attn = np.empty((B, N, H, K), np.float32)
    for c in range(NCORES):
        om = results[c]
        op = np.asarray(om["outp"])  # (2, 16384)
        output[c] = op.reshape(2, 64, 256).transpose(1, 0, 2).reshape(N, D)
        En = np.asarray(om["eraw"]).astype(np.float32)  # (32, 2048)
        # attn[n=4q+pp, hg, 8j+h'] = En[8pp+h', q*64 + 8*hg + j]
        E5 = En.reshape(4, 8, NQ, 8, 8)  # [pp, h', q, hg, j]
        attn[c] = E5.transpose(2, 0, 3, 4, 1).reshape(N, H, K)
    return output, attn


def kernel(q, k, v, W, b):
    from concourse.bass_utils import run_bass_kernel_spmd

    nc = _build()
    in_maps = _host_prep(
        np.asarray(q, np.float32),
        np.asarray(k, np.float32),
        np.asarray(v, np.float32),
        np.asarray(W, np.float32),
        np.asarray(b, np.float32),
    )
    res = run_bass_kernel_spmd(nc, in_maps, core_ids=list(range(NCORES)))
    return _host_post(res.results)
